# revision 20
# baseline (speedup 1.0000x reference)
"""NCC loss (local normalized cross-correlation, window 9^3) on 8 Trainium2
NeuronCores.

Reference: 5 channels [I, J, I^2, J^2, IJ] box-filtered (separable 9-tap mean,
SAME zero-pad) over a 192^3 volume; cc = sigma12^2/(sigma1^2*sigma2^2+eps);
output = 1 - mean(cc).

Sharding: depth axis. Core c computes output slices [24c, 24c+24), reading
mean-shifted bf16 inputs for padded slices [24c, 24c+32) of the (+4 both
ends) zero-padded volume. Host pre-applies the -0.5 mean shift (pads become
-0.5, the shifted zero sample), casts to bf16, interleaves targ|pred, and
duplicates the w overlap so rows arrive w-blocked: [t(2), wc(2), 128] where
wc0 = ext w 0..127 and wc1 = ext w 96..223 (last 24 are pad).

Per-core pipeline:
  load  : 4 z-slices per DMA into [h-part, 4, 512] bf16 tiles.
  prep  : squares + cross product -> blocked (wc, ch, 128) channel tiles
          (2 DVE ops/slice/h-tile); ch0/ch1 feed the H matmul from raw.
  H pass: banded matmuls (TensorE) accumulated over slices into PSUM
          (cumsum over D, 10 blocks of 128 = 2.5 banks per h-tile);
          bf16 snapshots to SBUF each slice (DVE for A, ACT for B).
  T pass: one batched x-bar DMA transpose per snapshot tile per z-pair
          (out 3D AP [128, 10 blocks, rows]) -> T2 [w-part, (half,blk,h)].
  W pass: out(oz) = bandW+ @ T[oz+8] + bandW- @ T[oz-1] accumulated in
          PSUM: the D window diff is folded into the matmul.
  cc    : elementwise DVE ops + one Ln (ACT; natural_log set stays
          resident; square/copy are in that set too); lnf stored per oz;
          Exp+accumulate deferred to a tail pass (2 ACT table loads total).
Host: 1 - sum(partials)/192^3.
"""

import sys

import numpy as np

sys.path.insert(0, "/opt/trn_rl_repo")

import contextlib

import concourse.bacc as bacc
import concourse.mybir as mybir
from concourse import tile
from concourse.bass_utils import run_bass_kernel_spmd
from concourse.tile import add_dep_helper

F32 = mybir.dt.float32
BF16 = mybir.dt.bfloat16
AOT = mybir.AluOpType
ACTF = mybir.ActivationFunctionType
AXL = mybir.AxisListType

H = 192
W = 192
D_TOT = 192
HE = 200
PAD = 4
N_CORES = 8

HA = 112           # h-tile A: out rows 0..111 (ext rows 4..115)
HB = 80            # h-tile B: out rows 112..191 (ext rows 116..195)
KT = 128           # A contraction rows: ext h 0..127 (uses 0..119)
KB = 88            # B contraction rows: ext h 112..199

NBLK = 10          # (wc, ch) blocks of 128 cols
BLKW = 128
VALW = 104         # valid w cols per block
ROWW = 512         # raw row: t(2) x wc(2) x 128
TW = NBLK * H      # 1920: T2 free size per z half
TP = 480           # W matmul piece width (4 pieces)

ZB = 4             # z slices per input DMA batch
NPAIR = 3          # snapshot pair ring
NT2 = 6            # transposed z-pair ring

BAND_C = 1.0 / 27.0
EPS = float(np.finfo(np.float32).eps)
TINY = float(np.finfo(np.float32).tiny)


def _band(rows, cols, lo, hi, val):
    k = np.arange(rows)[:, None]
    m = np.arange(cols)[None, :]
    return np.where((k - m >= lo) & (k - m <= hi), val, 0.0).astype(np.float32)


def make_consts():
    import ml_dtypes

    # [120, 304]: cols 0:112 = H band; 112:208 = +W band; 208:304 = -W band
    b = np.zeros((120, 304), np.float32)
    b[:, 0:112] = _band(120, 112, 0, 8, BAND_C)
    bw = _band(104, 96, 0, 8, BAND_C)
    b[0:104, 112:208] = bw
    b[0:104, 208:304] = -bw
    return b.astype(ml_dtypes.bfloat16)


def build_program(din, dout, dbg=False):
    assert din == dout + 2 * PAD
    nc = bacc.Bacc(
        "TRN2", target_bir_lowering=False, debug=False, num_devices=N_CORES
    )

    raw_d = nc.dram_tensor(
        "raw", [din, HE, 2, 2, BLKW], BF16, kind="ExternalInput"
    )
    band_d = nc.dram_tensor("band", [120, 304], BF16, kind="ExternalInput")
    out_d = nc.dram_tensor("out", [96, 1], F32, kind="ExternalOutput")
    if dbg:
        dbg_sA = nc.dram_tensor("dbg_sA", [HA, 2, NBLK * BLKW], BF16,
                                kind="ExternalOutput")
        dbg_t2 = nc.dram_tensor("dbg_t2", [128, 2, TW], BF16,
                                kind="ExternalOutput")
        dbg_ff = nc.dram_tensor("dbg_ff", [2, 96, 4 * TP], BF16,
                                kind="ExternalOutput")

    raw = raw_d.ap()

    with tile.TileContext(nc) as tc, contextlib.ExitStack() as ctx:
        consts = ctx.enter_context(tc.tile_pool(name="consts", bufs=1))
        raws = ctx.enter_context(tc.tile_pool(name="raws", bufs=2))
        chans = ctx.enter_context(tc.tile_pool(name="chans", bufs=3))
        snaps = ctx.enter_context(tc.tile_pool(name="snaps", bufs=1))
        t2s = ctx.enter_context(tc.tile_pool(name="t2s", bufs=1))
        ffs = ctx.enter_context(tc.tile_pool(name="ffs", bufs=3))
        dts = ctx.enter_context(tc.tile_pool(name="dts", bufs=3))
        ccs = ctx.enter_context(tc.tile_pool(name="ccs", bufs=3))
        accp = ctx.enter_context(tc.tile_pool(name="accp", bufs=1))
        ps_h = ctx.enter_context(tc.tile_pool(name="psh", bufs=1, space="PSUM"))
        ps_w = ctx.enter_context(tc.tile_pool(name="psw", bufs=1, space="PSUM"))

        band = consts.tile([120, 304], BF16, tag="band")
        nc.sync.dma_start(band[:], band_d.ap())
        bandH_A = band[0:120, 0:112]
        bandH_B = band[0:88, 0:80]
        bandW_p = band[0:104, 112:208]
        bandW_n = band[0:104, 208:304]

        bias_tiny = consts.tile([128, 1], F32, tag="bias_tiny")
        nc.vector.memset(bias_tiny[:], TINY)

        acc = accp.tile([96, dout], F32, tag="acc")
        nc.vector.memset(acc[:], 0.0)
        lnf_buf = accp.tile([96, dout, 2 * H], BF16, tag="lnf")

        # H cumsum PSUM: 10 blocks of 128 -> 3 banks per h-tile
        psA = ps_h.tile([HA, 1536], F32, tag="psA")
        psB = ps_h.tile([HB, 1536], F32, tag="psB")
        ps3A = psA.rearrange("p (b w) -> p b w", b=3)
        ps3B = psB.rearrange("p (b w) -> p b w", b=3)

        # W PSUM: 2 x 1 bank, each used twice per oz
        pws = [
            ps_w.tile([96, 512], F32, tag=f"pw{i}", name=f"pw{i}")
            for i in range(2)
        ]

        # persistent snapshot pair tiles (memset once: pad cols stay 0)
        sAp, sBp = [], []
        for i in range(NPAIR):
            a = snaps.tile([HA, 2, NBLK * BLKW], BF16, tag=f"sAp{i}",
                           name=f"sAp{i}")
            b = snaps.tile([HB, 2, NBLK * BLKW], BF16, tag=f"sBp{i}",
                           name=f"sBp{i}")
            nc.vector.memset(a[:], 0.0)
            nc.vector.memset(b[:], 0.0)
            sAp.append(a)
            sBp.append(b)

        # persistent transposed tiles [w 128, (half, blk, h)]
        T2 = [
            t2s.tile([128, 2, TW], BF16, tag=f"T2_{i}", name=f"T2_{i}")
            for i in range(NT2)
        ]

        raw_tiles = {}

        def load_batch(zb):
            rT = raws.tile([KT, ZB, ROWW], BF16, tag="rT", name="rT")
            rB = raws.tile([KB, ZB, ROWW], BF16, tag="rB", name="rB")
            z0 = zb * ZB
            nc.sync.dma_start(
                rT[:],
                raw[z0 : z0 + ZB, 0:KT].rearrange(
                    "z h t wc w -> h z t wc w"
                ),
            )
            nc.sync.dma_start(
                rB[:],
                raw[z0 : z0 + ZB, HE - KB : HE].rearrange(
                    "z h t wc w -> h z t wc w"
                ),
            )
            raw_tiles[zb] = (rT, rB)

        def prep(z):
            rT, rB = raw_tiles[z // ZB]
            zs = z % ZB
            cT = chans.tile([KT, 6 * BLKW], BF16, tag="cT", name="cT")
            cB = chans.tile([KB, 6 * BLKW], BF16, tag="cB", name="cB")
            for ch, r in ((cT, rT), (cB, rB)):
                # [p, wc, t, w] view of this z slice
                rv = r[:, zs, :].rearrange(
                    "p (t wc w) -> p wc t w", t=2, wc=2
                )
                c4 = ch.rearrange("p (wc b w) -> p wc b w", wc=2, b=3)
                # ch2 = targ^2, ch3 = pred^2 (both wc at once)
                nc.gpsimd.tensor_tensor(
                    c4[:, :, 0:2, 0:VALW],
                    rv[:, :, :, 0:VALW],
                    rv[:, :, :, 0:VALW],
                    AOT.mult,
                )
                # ch4 = targ * pred
                nc.gpsimd.tensor_tensor(
                    c4[:, :, 2, 0:VALW],
                    rv[:, :, 0, 0:VALW],
                    rv[:, :, 1, 0:VALW],
                    AOT.mult,
                )
            return cT, cB

        def h_pass(z, cT, cB):
            # One matmul per PSUM bank: start=True clears has_written at
            # bank granularity, so each bank must be a single accum group.
            # Resulting block order: 0:(t,wc0) 1:(t,wc1) 2:(p,wc0) 3:(p,wc1)
            # 4:(t2,wc0) 5:(p2,wc0) 6:(tp,wc0) 7:(t2,wc1) 8:(p2,wc1) 9:(tp,wc1)
            rT, rB = raw_tiles[z // ZB]
            zs = z % ZB
            start = z == 0
            kw = dict(start=start, stop=True, skip_group_check=True)
            mm = nc.tensor.matmul
            for (ps3, bH, r, ch, kk) in (
                (ps3A, bandH_A, rT, cT, 120),
                (ps3B, bandH_B, rB, cB, 88),
            ):
                rv = r[:, zs, :].rearrange(
                    "p (t wc w) -> p t wc w", t=2, wc=2
                )
                mm(ps3[:, 0, 0:512], bH, rv[0:kk], **kw)
                mm(ps3[:, 1, 0:512], bH, ch[0:kk, 0:512], **kw)
                mm(ps3[:, 2, 0:256], bH, ch[0:kk, 512:768], **kw)

        def snapshot(z):
            # per-PSUM-bank copies so each bank is released for the next
            # slice's matmul as soon as its own copy completes
            half = z % 2
            pa = sAp[(z // 2) % NPAIR]
            pb = sBp[(z // 2) % NPAIR]
            srcA = psA.rearrange("p (b w) -> p b w", b=12)
            srcB = psB.rearrange("p (b w) -> p b w", b=12)
            dA = pa.rearrange("p h (b w) -> p h b w", b=NBLK)
            dB = pb.rearrange("p h (b w) -> p h b w", b=NBLK)
            for b0, b1 in ((0, 4), (4, 8), (8, 10)):
                nc.vector.tensor_copy(
                    dA[:, half, b0:b1, 0:VALW], srcA[:, b0:b1, 0:VALW]
                )
                nc.scalar.copy(
                    dB[:, half, b0:b1, 0:VALW], srcB[:, b0:b1, 0:VALW]
                )

        def transpose_pair(z):
            # after snapshot of odd z: pair (z//2) holds z-1, z
            p = z // 2
            t2 = T2[p % NT2]
            pa, pb = sAp[p % NPAIR], sBp[p % NPAIR]
            t4 = t2.rearrange("p h (b q) -> p h b q", b=NBLK)
            for half in range(2):
                nc.sync.dma_start_transpose(
                    t4[:, half, :, 0:HA], pa[:, half, :]
                )
                nc.sync.dma_start_transpose(
                    t4[:, half, :, HA:H], pb[:, half, :]
                )

        def w_pass(oz):
            # D window diff on the transposed snapshots (off the critical
            # chain: inputs are ready ~8 slices before the W matmuls run),
            # then a single banded matmul per piece.
            zh = oz + 8
            hi = T2[(zh // 2) % NT2][0:104, zh % 2, :]
            if oz > 0:
                zl = oz - 1
                lo = T2[(zl // 2) % NT2][0:104, zl % 2, :]
                dT = dts.tile([104, TW], BF16, tag="dT", name="dT")
                nc.vector.tensor_tensor(dT[:], hi, lo, AOT.subtract)
                rhs = dT[:]
            else:
                rhs = hi

            ff = ffs.tile([96, 4 * TP], BF16, tag="ff", name="ff")
            for pp in range(2):        # piece pairs (0,1) then (2,3)
                for q in range(2):
                    sl = slice((2 * pp + q) * TP, (2 * pp + q + 1) * TP)
                    nc.tensor.matmul(
                        pws[q][:, 0:TP], bandW_p, rhs[:, sl],
                        start=True, stop=True,
                    )
                sl0 = slice(2 * pp * TP, (2 * pp + 1) * TP)
                sl1 = slice((2 * pp + 1) * TP, (2 * pp + 2) * TP)
                nc.vector.tensor_copy(ff[:, sl0], pws[0][:, 0:TP])
                nc.scalar.copy(ff[:, sl1], pws[1][:, 0:TP])

            if dbg and oz <= 1:
                nc.sync.dma_start(dbg_ff.ap()[oz], ff[:])
            cc(oz, ff)

        def cc(oz, ff):
            # block order (see h_pass): mu pair = blocks 0..3 (ch-major,
            # wc-minor); conv trio = blocks 4..9 (wc-major, ch-minor)
            mus = ff[:, 0 : 4 * H].rearrange("p (c wc h) -> p c wc h", c=2,
                                             wc=2)
            cnv = ff[:, 4 * H :].rearrange("p (wc c h) -> p c wc h", wc=2,
                                           c=3)
            mu1 = mus[:, 0]
            mu2 = mus[:, 1]
            cIJ = cnv[:, 2]
            sc = ccs.tile([96, 2, 2, H], BF16, tag="sc", name="sc")
            var = ccs.tile([96, 2, 2, H], BF16, tag="var", name="var")
            lnp = ccs.tile([96, 2, 2, H], BF16, tag="lnp", name="lnp")
            lno = ccs.tile([96, 2, 2, H], BF16, tag="lno", name="lno")
            sqs = ccs.tile([96, 2, 2, H], BF16, tag="sqs", name="sqs")
            t1 = sc[:, 0]
            s12 = sc[:, 1]

            # t1 = mu1*mu2 ; s12 = F_IJ - t1
            nc.vector.tensor_tensor(t1, mu1, mu2, AOT.mult)
            nc.vector.tensor_tensor(s12, cIJ, t1, AOT.subtract)
            # sqs = [mu1^2, mu2^2] on ACT (square is in the natural_log
            # set -> no table switch); var = conv - sqs   (dims (ch, wc))
            nc.scalar.activation(sqs[:], mus[:], ACTF.Square)
            nc.vector.tensor_tensor(var[:], cnv[:, 0:2], sqs[:],
                                    AOT.subtract)
            # lnp = [s12^2 | sg1*sg2]
            nc.vector.tensor_tensor(lnp[:, 0], s12, s12, AOT.mult)
            nc.vector.tensor_tensor(lnp[:, 1], var[:, 0], var[:, 1],
                                    AOT.mult)
            ln_insts.append(
                nc.scalar.activation(lno[:], lnp[:], ACTF.Ln,
                                     bias=bias_tiny[0:96, :])
            )
            lv = lnf_buf[:, oz, :].rearrange("p (wc h) -> p wc h", wc=2)
            nc.vector.tensor_tensor(lv, lno[:, 0], lno[:, 1], AOT.subtract)

        ln_insts = []
        for z in range(din):
            if z % ZB == 0:
                load_batch(z // ZB)
            cT, cB = prep(z)
            h_pass(z, cT, cB)
            snapshot(z)
            if z % 2 == 1:
                transpose_pair(z)
                if dbg and z == 1:
                    nc.sync.dma_start(dbg_sA.ap(), sAp[0][:])
                    nc.sync.dma_start(dbg_t2.ap(), T2[0][:])
                if z >= 9:
                    w_pass(z - 9)
                    w_pass(z - 8)

        # tail: all Exps (one table switch), accumulate per-oz sums.
        # Explicit dep on the last Ln so the scheduler cannot interleave
        # Exps into the loop (which would ping-pong the ACT table set).
        ccout = accp.tile([96, 2 * H], BF16, tag="ccout")
        for oz in range(dout):
            e = nc.scalar.activation(
                ccout[:], lnf_buf[:, oz, :], ACTF.Exp,
                accum_out=acc[:, oz : oz + 1],
            )
            if oz == 0:
                add_dep_helper(e.ins, ln_insts[-1].ins, sync=True,
                               reason="defer exps past all lns")
        accv = accp.tile([96, 1], F32, tag="accv")
        nc.vector.tensor_reduce(accv[:], acc[:], AXL.X, AOT.add)
        nc.sync.dma_start(out_d.ap(), accv[:])

    _dedup_ldweights(nc)
    nc.compile()
    return nc


def _dedup_ldweights(nc):
    """Remove InstLdweights whose weights are already loaded.

    Tile legalization emits one LDWEIGHTS per matmul even when consecutive
    matmuls share the stationary operand. PE executes LDW/MM in program
    order (the reorder window preserves semantics), so a repeated load of
    identical weights is a pure ~200ns PE stall. Waits on a removed load
    are merged into the next PE instruction.
    """
    for blk in nc.main_func.blocks:
        insts = blk.instructions
        cur_sig = None
        to_remove = []
        pending_waits = []
        for ins in list(insts):
            if isinstance(ins, mybir.InstLdweights):
                si = ins.sync_info
                if si is not None and len(si.on_update) > 0:
                    cur_sig = None  # keep: it signals someone
                    continue
                sig = (
                    str(ins.ins[0]),
                    str(ins.is_transpose),
                    str(ins.perf_mode),
                    str(ins.tile_position),
                    str(ins.tile_size),
                )
                if sig == cur_sig:
                    to_remove.append(ins)
                    if si is not None and len(si.on_wait) > 0:
                        pending_waits.extend(si.on_wait)
                else:
                    cur_sig = sig
            elif ins.engine == mybir.EngineType.PE and ins.is_executable():
                if pending_waits:
                    si = ins.sync_info
                    if si is None:
                        ins.sync_info = mybir.SyncInfo(
                            on_wait=list(pending_waits), on_update=[]
                        )
                    else:
                        ins.sync_info = mybir.SyncInfo(
                            on_wait=list(si.on_wait) + list(pending_waits),
                            on_update=list(si.on_update),
                        )
                    pending_waits = []
        assert not pending_waits
        for ins in to_remove:
            insts.remove(ins)


_PROGRAM_CACHE = {}


def _get_program(din, dout):
    key = (din, dout)
    if key not in _PROGRAM_CACHE:
        _PROGRAM_CACHE[key] = build_program(din, dout)
    return _PROGRAM_CACHE[key]


def make_in_maps(pred, targ):
    """Build per-core input maps from full 192^3 f32 volumes."""
    import ml_dtypes

    dout = D_TOT // N_CORES
    din = dout + 2 * PAD

    # shifted, padded volume rows: [dpad, 200, 2, 224] bf16, pads = -0.5
    dpad = D_TOT + 2 * PAD
    we = np.full((dpad, HE, 2, 224), -0.5, np.float32)
    we[PAD:-PAD, PAD : PAD + H, 0, PAD : PAD + W] = targ - 0.5
    we[PAD:-PAD, PAD : PAD + H, 1, PAD : PAD + W] = pred - 0.5
    # w-blocked rows: [dpad, 200, 2, 2, 128]
    arr = np.empty((dpad, HE, 2, 2, BLKW), np.float32)
    arr[:, :, :, 0, :] = we[:, :, :, 0:128]
    arr[:, :, :, 1, :] = we[:, :, :, 96:224]
    arr = arr.astype(ml_dtypes.bfloat16)

    band = make_consts()
    in_maps = []
    for c in range(N_CORES):
        s = c * dout
        in_maps.append(
            {
                "raw": np.ascontiguousarray(arr[s : s + din]),
                "band": band,
            }
        )
    return in_maps


def kernel(pred, target):
    pred = np.asarray(pred).reshape(D_TOT, H, W).astype(np.float32)
    targ = np.asarray(target).reshape(D_TOT, H, W).astype(np.float32)

    dout = D_TOT // N_CORES
    din = dout + 2 * PAD

    nc = _get_program(din, dout)
    in_maps = make_in_maps(pred, targ)

    res = run_bass_kernel_spmd(nc, in_maps, core_ids=list(range(N_CORES)))
    total = sum(float(r["out"].astype(np.float64).sum()) for r in res.results)
    return np.float32(1.0 - total / float(D_TOT * H * W))


# revision 23
# speedup vs baseline: 1.0786x; 1.0786x over previous
"""NCC loss (local normalized cross-correlation, window 9^3) on 8 Trainium2
NeuronCores.

Reference: 5 channels [I, J, I^2, J^2, IJ] box-filtered (separable 9-tap mean,
SAME zero-pad) over a 192^3 volume; cc = sigma12^2/(sigma1^2*sigma2^2+eps);
output = 1 - mean(cc).

Sharding: depth axis. Core c computes output slices [24c, 24c+24), reading
mean-shifted bf16 inputs for padded slices [24c, 24c+32) of the (+4 both
ends) zero-padded volume. Host pre-applies the -0.5 mean shift (pads become
-0.5, the shifted zero sample), casts to bf16, interleaves targ|pred, and
duplicates the w overlap so rows arrive w-blocked: [t(2), wc(2), 128] where
wc0 = ext w 0..127 and wc1 = ext w 96..223 (last 24 are pad).

Per-core pipeline:
  load  : 4 z-slices per DMA into [h-part, 4, 512] bf16 tiles.
  prep  : squares + cross product -> blocked (wc, ch, 128) channel tiles
          (2 DVE ops/slice/h-tile); ch0/ch1 feed the H matmul from raw.
  H pass: banded matmuls (TensorE) accumulated over slices into PSUM
          (cumsum over D, 10 blocks of 128 = 2.5 banks per h-tile);
          bf16 snapshots to SBUF each slice (DVE for A, ACT for B).
  T pass: one batched x-bar DMA transpose per snapshot tile per z-pair
          (out 3D AP [128, 10 blocks, rows]) -> T2 [w-part, (half,blk,h)].
  W pass: out(oz) = bandW+ @ T[oz+8] + bandW- @ T[oz-1] accumulated in
          PSUM: the D window diff is folded into the matmul.
  cc    : elementwise DVE ops + one Ln (ACT; natural_log set stays
          resident; square/copy are in that set too); lnf stored per oz;
          Exp+accumulate deferred to a tail pass (2 ACT table loads total).
Host: 1 - sum(partials)/192^3.
"""

import sys

import numpy as np

sys.path.insert(0, "/opt/trn_rl_repo")

import contextlib

import concourse.bacc as bacc
import concourse.mybir as mybir
from concourse import tile
from concourse.bass_utils import run_bass_kernel_spmd
from concourse.tile import add_dep_helper

F32 = mybir.dt.float32
BF16 = mybir.dt.bfloat16
AOT = mybir.AluOpType
ACTF = mybir.ActivationFunctionType
AXL = mybir.AxisListType

H = 192
W = 192
D_TOT = 192
HE = 200
PAD = 4
N_CORES = 8

HA = 112           # h-tile A: out rows 0..111 (ext rows 4..115)
HB = 80            # h-tile B: out rows 112..191 (ext rows 116..195)
KT = 128           # A contraction rows: ext h 0..127 (uses 0..119)
KB = 88            # B contraction rows: ext h 112..199

NBLK = 10          # (wc, ch) blocks of 128 cols
BLKW = 128
VALW = 104         # valid w cols per block
ROWW = 512         # raw row: t(2) x wc(2) x 128
TW = NBLK * H      # 1920: T2 free size per z half
TP = 480           # W matmul piece width (4 pieces)

ZB = 4             # z slices per input DMA batch
NPAIR = 3          # snapshot pair ring
NT2 = 6            # transposed z-pair ring

BAND_C = 1.0 / 27.0
EPS = float(np.finfo(np.float32).eps)
TINY = float(np.finfo(np.float32).tiny)


def _band(rows, cols, lo, hi, val):
    k = np.arange(rows)[:, None]
    m = np.arange(cols)[None, :]
    return np.where((k - m >= lo) & (k - m <= hi), val, 0.0).astype(np.float32)


def make_consts():
    import ml_dtypes

    # [120, 304]: cols 0:112 = H band; 112:208 = +W band; 208:304 = -W band
    b = np.zeros((120, 304), np.float32)
    b[:, 0:112] = _band(120, 112, 0, 8, BAND_C)
    bw = _band(104, 96, 0, 8, BAND_C)
    b[0:104, 112:208] = bw
    b[0:104, 208:304] = -bw
    return b.astype(ml_dtypes.bfloat16)


def build_program(din, dout, dbg=False):
    assert din == dout + 2 * PAD
    nc = bacc.Bacc(
        "TRN2", target_bir_lowering=False, debug=False, num_devices=N_CORES
    )

    raw_d = nc.dram_tensor(
        "raw", [din, HE, 2, 2, BLKW], BF16, kind="ExternalInput"
    )
    band_d = nc.dram_tensor("band", [120, 304], BF16, kind="ExternalInput")
    out_d = nc.dram_tensor("out", [96, 1], F32, kind="ExternalOutput")
    if dbg:
        dbg_sA = nc.dram_tensor("dbg_sA", [HA, 2, NBLK * BLKW], BF16,
                                kind="ExternalOutput")
        dbg_t2 = nc.dram_tensor("dbg_t2", [128, 2, TW], BF16,
                                kind="ExternalOutput")
        dbg_ff = nc.dram_tensor("dbg_ff", [2, 96, 4 * TP], BF16,
                                kind="ExternalOutput")

    raw = raw_d.ap()

    with tile.TileContext(nc) as tc, contextlib.ExitStack() as ctx:
        consts = ctx.enter_context(tc.tile_pool(name="consts", bufs=1))
        raws = ctx.enter_context(tc.tile_pool(name="raws", bufs=2))
        chans = ctx.enter_context(tc.tile_pool(name="chans", bufs=3))
        snaps = ctx.enter_context(tc.tile_pool(name="snaps", bufs=1))
        t2s = ctx.enter_context(tc.tile_pool(name="t2s", bufs=1))
        ffs = ctx.enter_context(tc.tile_pool(name="ffs", bufs=3))
        dts = ctx.enter_context(tc.tile_pool(name="dts", bufs=3))
        ccs = ctx.enter_context(tc.tile_pool(name="ccs", bufs=3))
        accp = ctx.enter_context(tc.tile_pool(name="accp", bufs=1))
        ps_h = ctx.enter_context(tc.tile_pool(name="psh", bufs=1, space="PSUM"))
        ps_w = ctx.enter_context(tc.tile_pool(name="psw", bufs=1, space="PSUM"))

        band = consts.tile([120, 304], BF16, tag="band")
        nc.sync.dma_start(band[:], band_d.ap())
        bandH_A = band[0:120, 0:112]
        bandH_B = band[0:88, 0:80]
        bandW_p = band[0:104, 112:208]
        bandW_n = band[0:104, 208:304]

        bias_tiny = consts.tile([128, 1], F32, tag="bias_tiny")
        nc.vector.memset(bias_tiny[:], TINY)

        acc = accp.tile([96, dout], F32, tag="acc")
        nc.vector.memset(acc[:], 0.0)
        lnf_buf = accp.tile([96, dout, 2 * H], BF16, tag="lnf")

        # H cumsum PSUM: 10 blocks of 128 -> 3 banks per h-tile
        psA = ps_h.tile([HA, 1536], F32, tag="psA")
        psB = ps_h.tile([HB, 1536], F32, tag="psB")
        ps3A = psA.rearrange("p (b w) -> p b w", b=3)
        ps3B = psB.rearrange("p (b w) -> p b w", b=3)

        # W PSUM: 2 x 1 bank, each used twice per oz
        pws = [
            ps_w.tile([96, 512], F32, tag=f"pw{i}", name=f"pw{i}")
            for i in range(2)
        ]

        # persistent snapshot pair tiles (memset once: pad cols stay 0)
        sAp, sBp = [], []
        for i in range(NPAIR):
            a = snaps.tile([HA, 2, NBLK * BLKW], BF16, tag=f"sAp{i}",
                           name=f"sAp{i}")
            b = snaps.tile([HB, 2, NBLK * BLKW], BF16, tag=f"sBp{i}",
                           name=f"sBp{i}")
            nc.vector.memset(a[:], 0.0)
            nc.vector.memset(b[:], 0.0)
            sAp.append(a)
            sBp.append(b)

        # persistent transposed tiles [w 128, (half, blk, h)]
        T2 = [
            t2s.tile([128, 2, TW], BF16, tag=f"T2_{i}", name=f"T2_{i}")
            for i in range(NT2)
        ]

        raw_tiles = {}

        def load_batch(zb):
            rT = raws.tile([KT, ZB, ROWW], BF16, tag="rT", name="rT")
            rB = raws.tile([KB, ZB, ROWW], BF16, tag="rB", name="rB")
            z0 = zb * ZB
            nc.sync.dma_start(
                rT[:],
                raw[z0 : z0 + ZB, 0:KT].rearrange(
                    "z h t wc w -> h z t wc w"
                ),
            )
            nc.sync.dma_start(
                rB[:],
                raw[z0 : z0 + ZB, HE - KB : HE].rearrange(
                    "z h t wc w -> h z t wc w"
                ),
            )
            raw_tiles[zb] = (rT, rB)

        def prep(z):
            rT, rB = raw_tiles[z // ZB]
            zs = z % ZB
            cT = chans.tile([KT, 6 * BLKW], BF16, tag="cT", name="cT")
            cB = chans.tile([KB, 6 * BLKW], BF16, tag="cB", name="cB")
            for ch, r in ((cT, rT), (cB, rB)):
                # [p, wc, t, w] view of this z slice
                rv = r[:, zs, :].rearrange(
                    "p (t wc w) -> p wc t w", t=2, wc=2
                )
                c4 = ch.rearrange("p (wc b w) -> p wc b w", wc=2, b=3)
                # ch2 = targ^2, ch3 = pred^2 (both wc at once)
                nc.gpsimd.tensor_tensor(
                    c4[:, :, 0:2, 0:VALW],
                    rv[:, :, :, 0:VALW],
                    rv[:, :, :, 0:VALW],
                    AOT.mult,
                )
                # ch4 = targ * pred
                nc.gpsimd.tensor_tensor(
                    c4[:, :, 2, 0:VALW],
                    rv[:, :, 0, 0:VALW],
                    rv[:, :, 1, 0:VALW],
                    AOT.mult,
                )
            return cT, cB

        def h_pass(z, cT, cB):
            # One matmul per PSUM bank: start=True clears has_written at
            # bank granularity, so each bank must be a single accum group.
            # Resulting block order: 0:(t,wc0) 1:(t,wc1) 2:(p,wc0) 3:(p,wc1)
            # 4:(t2,wc0) 5:(p2,wc0) 6:(tp,wc0) 7:(t2,wc1) 8:(p2,wc1) 9:(tp,wc1)
            rT, rB = raw_tiles[z // ZB]
            zs = z % ZB
            start = z == 0
            kw = dict(start=start, stop=True, skip_group_check=True)
            mm = nc.tensor.matmul
            for (ps3, bH, r, ch, kk) in (
                (ps3A, bandH_A, rT, cT, 120),
                (ps3B, bandH_B, rB, cB, 88),
            ):
                rv = r[:, zs, :].rearrange(
                    "p (t wc w) -> p t wc w", t=2, wc=2
                )
                mm(ps3[:, 0, 0:512], bH, rv[0:kk], **kw)
                mm(ps3[:, 1, 0:512], bH, ch[0:kk, 0:512], **kw)
                mm(ps3[:, 2, 0:256], bH, ch[0:kk, 512:768], **kw)

        def snapshot(z):
            # per-PSUM-bank copies so each bank is released for the next
            # slice's matmul as soon as its own copy completes
            half = z % 2
            pa = sAp[(z // 2) % NPAIR]
            pb = sBp[(z // 2) % NPAIR]
            srcA = psA.rearrange("p (b w) -> p b w", b=12)
            srcB = psB.rearrange("p (b w) -> p b w", b=12)
            dA = pa.rearrange("p h (b w) -> p h b w", b=NBLK)
            dB = pb.rearrange("p h (b w) -> p h b w", b=NBLK)
            for b0, b1 in ((0, 4), (4, 8), (8, 10)):
                nc.vector.tensor_copy(
                    dA[:, half, b0:b1, 0:VALW], srcA[:, b0:b1, 0:VALW]
                )
                nc.scalar.copy(
                    dB[:, half, b0:b1, 0:VALW], srcB[:, b0:b1, 0:VALW]
                )

        def transpose_z(z):
            # transpose this z's snapshot half right after its snapshot
            p = z // 2
            half = z % 2
            t2 = T2[p % NT2]
            pa, pb = sAp[p % NPAIR], sBp[p % NPAIR]
            t4 = t2.rearrange("p h (b q) -> p h b q", b=NBLK)
            nc.sync.dma_start_transpose(t4[:, half, :, 0:HA], pa[:, half, :])
            nc.sync.dma_start_transpose(t4[:, half, :, HA:H], pb[:, half, :])

        def w_pass(oz):
            zh = oz + 8
            hi = T2[(zh // 2) % NT2][0:104, zh % 2, :]
            lo = None
            if oz > 0:
                zl = oz - 1
                lo = T2[(zl // 2) % NT2][0:104, zl % 2, :]

            ff = ffs.tile([96, 4 * TP], BF16, tag="ff", name="ff")
            for pp in range(2):        # piece pairs (0,1) then (2,3)
                for q in range(2):
                    sl = slice((2 * pp + q) * TP, (2 * pp + q + 1) * TP)
                    nc.tensor.matmul(
                        pws[q][:, 0:TP], bandW_p, hi[:, sl],
                        start=True, stop=lo is None,
                    )
                if lo is not None:
                    for q in range(2):
                        sl = slice((2 * pp + q) * TP, (2 * pp + q + 1) * TP)
                        nc.tensor.matmul(
                            pws[q][:, 0:TP], bandW_n, lo[:, sl],
                            start=False, stop=True,
                        )
                sl0 = slice(2 * pp * TP, (2 * pp + 1) * TP)
                sl1 = slice((2 * pp + 1) * TP, (2 * pp + 2) * TP)
                nc.vector.tensor_copy(ff[:, sl0], pws[0][:, 0:TP])
                nc.scalar.copy(ff[:, sl1], pws[1][:, 0:TP])

            if dbg and oz <= 1:
                nc.sync.dma_start(dbg_ff.ap()[oz], ff[:])
            cc(oz, ff)

        def cc(oz, ff):
            # block order (see h_pass): mu pair = blocks 0..3 (ch-major,
            # wc-minor); conv trio = blocks 4..9 (wc-major, ch-minor)
            mus = ff[:, 0 : 4 * H].rearrange("p (c wc h) -> p c wc h", c=2,
                                             wc=2)
            cnv = ff[:, 4 * H :].rearrange("p (wc c h) -> p c wc h", wc=2,
                                           c=3)
            mu1 = mus[:, 0]
            mu2 = mus[:, 1]
            cIJ = cnv[:, 2]
            sc = ccs.tile([96, 2, 2, H], BF16, tag="sc", name="sc")
            var = ccs.tile([96, 2, 2, H], BF16, tag="var", name="var")
            lnp = ccs.tile([96, 2, 2, H], BF16, tag="lnp", name="lnp")
            lno = ccs.tile([96, 2, 2, H], BF16, tag="lno", name="lno")
            sqs = ccs.tile([96, 2, 2, H], BF16, tag="sqs", name="sqs")
            t1 = sc[:, 0]
            s12 = sc[:, 1]

            # t1 = mu1*mu2 ; s12 = F_IJ - t1
            nc.vector.tensor_tensor(t1, mu1, mu2, AOT.mult)
            nc.vector.tensor_tensor(s12, cIJ, t1, AOT.subtract)
            # sqs = [mu1^2, mu2^2] on ACT (square is in the natural_log
            # set -> no table switch); var = conv - sqs   (dims (ch, wc))
            nc.scalar.activation(sqs[:], mus[:], ACTF.Square)
            nc.vector.tensor_tensor(var[:], cnv[:, 0:2], sqs[:],
                                    AOT.subtract)
            # lnp = [s12^2 | sg1*sg2]
            nc.vector.tensor_tensor(lnp[:, 0], s12, s12, AOT.mult)
            nc.vector.tensor_tensor(lnp[:, 1], var[:, 0], var[:, 1],
                                    AOT.mult)
            ln_insts.append(
                nc.scalar.activation(lno[:], lnp[:], ACTF.Ln,
                                     bias=bias_tiny[0:96, :])
            )
            lv = lnf_buf[:, oz, :].rearrange("p (wc h) -> p wc h", wc=2)
            nc.vector.tensor_tensor(lv, lno[:, 0], lno[:, 1], AOT.subtract)

        ln_insts = []
        for z in range(din):
            if z % ZB == 0:
                load_batch(z // ZB)
            cT, cB = prep(z)
            h_pass(z, cT, cB)
            snapshot(z)
            transpose_z(z)
            if dbg and z == 1:
                nc.sync.dma_start(dbg_sA.ap(), sAp[0][:])
                nc.sync.dma_start(dbg_t2.ap(), T2[0][:])
            if z >= 8:
                w_pass(z - 8)

        # tail: all Exps (one table switch), accumulate per-oz sums.
        # Explicit dep on the last Ln so the scheduler cannot interleave
        # Exps into the loop (which would ping-pong the ACT table set).
        ccout = accp.tile([96, 2 * H], BF16, tag="ccout")
        for oz in range(dout):
            e = nc.scalar.activation(
                ccout[:], lnf_buf[:, oz, :], ACTF.Exp,
                accum_out=acc[:, oz : oz + 1],
            )
            if oz == 0:
                add_dep_helper(e.ins, ln_insts[-1].ins, sync=True,
                               reason="defer exps past all lns")
        accv = accp.tile([96, 1], F32, tag="accv")
        nc.vector.tensor_reduce(accv[:], acc[:], AXL.X, AOT.add)
        nc.sync.dma_start(out_d.ap(), accv[:])

    _dedup_ldweights(nc)
    nc.compile()
    return nc


def _dedup_ldweights(nc):
    """Remove InstLdweights whose weights are already loaded.

    Tile legalization emits one LDWEIGHTS per matmul even when consecutive
    matmuls share the stationary operand. PE executes LDW/MM in program
    order (the reorder window preserves semantics), so a repeated load of
    identical weights is a pure ~200ns PE stall. Waits on a removed load
    are merged into the next PE instruction.
    """
    for blk in nc.main_func.blocks:
        insts = blk.instructions
        cur_sig = None
        to_remove = []
        pending_waits = []
        for ins in list(insts):
            if isinstance(ins, mybir.InstLdweights):
                si = ins.sync_info
                if si is not None and len(si.on_update) > 0:
                    cur_sig = None  # keep: it signals someone
                    continue
                sig = (
                    str(ins.ins[0]),
                    str(ins.is_transpose),
                    str(ins.perf_mode),
                    str(ins.tile_position),
                    str(ins.tile_size),
                )
                if sig == cur_sig:
                    to_remove.append(ins)
                    if si is not None and len(si.on_wait) > 0:
                        pending_waits.extend(si.on_wait)
                else:
                    cur_sig = sig
            elif ins.engine == mybir.EngineType.PE and ins.is_executable():
                if pending_waits:
                    si = ins.sync_info
                    if si is None:
                        ins.sync_info = mybir.SyncInfo(
                            on_wait=list(pending_waits), on_update=[]
                        )
                    else:
                        ins.sync_info = mybir.SyncInfo(
                            on_wait=list(si.on_wait) + list(pending_waits),
                            on_update=list(si.on_update),
                        )
                    pending_waits = []
        assert not pending_waits
        for ins in to_remove:
            insts.remove(ins)


_PROGRAM_CACHE = {}


def _get_program(din, dout):
    key = (din, dout)
    if key not in _PROGRAM_CACHE:
        _PROGRAM_CACHE[key] = build_program(din, dout)
    return _PROGRAM_CACHE[key]


def make_in_maps(pred, targ):
    """Build per-core input maps from full 192^3 f32 volumes."""
    import ml_dtypes

    dout = D_TOT // N_CORES
    din = dout + 2 * PAD

    # shifted, padded volume rows: [dpad, 200, 2, 224] bf16, pads = -0.5
    dpad = D_TOT + 2 * PAD
    we = np.full((dpad, HE, 2, 224), -0.5, np.float32)
    we[PAD:-PAD, PAD : PAD + H, 0, PAD : PAD + W] = targ - 0.5
    we[PAD:-PAD, PAD : PAD + H, 1, PAD : PAD + W] = pred - 0.5
    # w-blocked rows: [dpad, 200, 2, 2, 128]
    arr = np.empty((dpad, HE, 2, 2, BLKW), np.float32)
    arr[:, :, :, 0, :] = we[:, :, :, 0:128]
    arr[:, :, :, 1, :] = we[:, :, :, 96:224]
    arr = arr.astype(ml_dtypes.bfloat16)

    band = make_consts()
    in_maps = []
    for c in range(N_CORES):
        s = c * dout
        in_maps.append(
            {
                "raw": np.ascontiguousarray(arr[s : s + din]),
                "band": band,
            }
        )
    return in_maps


def kernel(pred, target):
    pred = np.asarray(pred).reshape(D_TOT, H, W).astype(np.float32)
    targ = np.asarray(target).reshape(D_TOT, H, W).astype(np.float32)

    dout = D_TOT // N_CORES
    din = dout + 2 * PAD

    nc = _get_program(din, dout)
    in_maps = make_in_maps(pred, targ)

    res = run_bass_kernel_spmd(nc, in_maps, core_ids=list(range(N_CORES)))
    total = sum(float(r["out"].astype(np.float64).sum()) for r in res.results)
    return np.float32(1.0 - total / float(D_TOT * H * W))
